# revision 70
# baseline (speedup 1.0000x reference)
"""Trainium2 Bass kernel for 16-head MultiHeadAttention (B=2, T=2048, D=1024).

Sharding (8 NeuronCores): core c handles batch b = c//4 and head group
g = c%4 (heads 4g..4g+3).  Each core computes Q/K/V projections for its 4
heads, attention, and a partial output projection against its 256 rows of
W_O.  The host sums the 4 partials per batch and adds b_O (row-parallel TP;
the all-reduce is folded into the unshard step).

Device layout notes:
 - The host pre-transposes x to x^T [D, T] so the contraction dim (features)
   lands on SBUF partitions without any on-device transposes of x.
 - Attention is computed in the S^T = K @ Q^T orientation: the softmax
   denominator is then a partition-axis sum, which the PE produces for free
   via a ones-column appended to V (out = [V|1]^T @ P^T gives O^T rows 0..63
   and the denominator in row 64).
 - Per head pair (2 heads of 64), weights are stacked to fill 128 partitions.
 - Matmul operands are bf16 (fp32 PSUM accumulation).

Schedule (v2): the kernel is ACT-bound on the 128 exp instructions
(~134us); everything else is arranged to keep that stream gap-free.
 - x_from DMA streams first; all four K/V tt0 projections consume feature
   chunks as they land (4 persistent psums = all 8 banks via the two
   pool tags).  tt1 + Q projections follow from resident x.
 - PSUM: tag "s" (2 bufs) rotates score tiles / projection tiles /
   out-projection tiles; tag "acc" (2 bufs) holds the double-buffered
   PV accumulators so stripe boundaries don't stall.
 - Softmax reciprocals use the ~5x reciprocal_approx_fast DVE op; the
   partition broadcast of 1/den and all PSUM->SBUF drains that used to sit
   on the Scalar engine moved to GpSimd, leaving Scalar pure-exp.
 - Out-projection t-chunks are pushed as atomic filler thunks into the
   hp1 attention stripes.
"""

import os
import sys

import numpy as np

for _p in ("/opt/trn_rl_repo", "/root/.axon_site/_ro/trn_rl_repo"):
    if os.path.isdir(_p) and _p not in sys.path:
        sys.path.insert(0, _p)

import concourse.bass as bass
import concourse.mybir as mybir
import concourse.tile as tile
from concourse import bacc
from concourse.bass_utils import run_bass_kernel_spmd
from concourse.masks import make_identity

F32 = mybir.dt.float32
BF16 = mybir.dt.bfloat16
AF = mybir.ActivationFunctionType

B, TQ, TK = 2, 2048, 2048
D = 1024          # model dim == x_to/x_from feature dim
H, DH = 16, 64
N_CORES = 8
HEADS_PER_CORE = 4   # one batch per core
HP = 2               # head pairs per core (2 heads of 64 stacked -> 128)

TT = 1024            # projection tile width (queries/keys)
N_TT = TQ // TT      # 2
N_SC = TK // 128     # 16 s-chunks
N_FC = D // 128      # 8 f-chunks
TA = 512             # attention stripe width (queries)

DT = BF16

_CACHED = {}


def build_program():
    nc = bacc.Bacc(
        "TRN2", target_bir_lowering=False, debug=False, num_devices=N_CORES
    )

    xt_to = nc.dram_tensor("xt_to", [D, TQ], DT, kind="ExternalInput")
    xt_from = nc.dram_tensor("xt_from", [D, TK], DT, kind="ExternalInput")
    # weights host-pre-arranged to [p, c, d] so each partition's DMA read
    # is one contiguous 4KB descriptor (the on-the-fly rearrange was 8x
    # 512B descriptors per partition and starved the early x stream)
    wq = nc.dram_tensor("wq", [128, N_FC, 256], DT, kind="ExternalInput")
    wk = nc.dram_tensor("wk", [128, N_FC, 256], DT, kind="ExternalInput")
    wv = nc.dram_tensor("wv", [128, N_FC, 256], DT, kind="ExternalInput")
    bq = nc.dram_tensor("bq", [128, 2], F32, kind="ExternalInput")
    bk = nc.dram_tensor("bk", [128, 2], F32, kind="ExternalInput")
    bv = nc.dram_tensor("bv", [128, 2], F32, kind="ExternalInput")
    wot = nc.dram_tensor("wot", [128, 2, 1024], DT, kind="ExternalInput")
    out = nc.dram_tensor("out", [TQ, D], DT, kind="ExternalOutput")

    xt_to_r = xt_to.rearrange("(c p) t -> p c t", p=128)
    xt_from_r = xt_from.rearrange("(c p) t -> p c t", p=128)

    with tile.TileContext(nc) as tc:
        with (
            tc.tile_pool(name="wpool", bufs=1) as wpool,
            tc.tile_pool(name="actpool", bufs=1) as actpool,
            tc.tile_pool(name="ptpool", bufs=8) as ptpool,
            tc.tile_pool(name="misc", bufs=2) as misc,
            tc.tile_pool(name="ps", bufs=2, space="PSUM") as pspool,
        ):
            # ---- constants & persistent tiles ----------------------------
            ident = wpool.tile([128, 128], DT)
            make_identity(nc, ident[:])

            wq_sb = wpool.tile([128, N_FC, 256], DT)
            wk_sb = wpool.tile([128, N_FC, 256], DT)
            wv_sb = wpool.tile([128, N_FC, 256], DT)
            bq_sb = wpool.tile([128, 2], F32)
            bk_sb = wpool.tile([128, 2], F32)
            bv_sb = wpool.tile([128, 2], F32)
            wot_sb = wpool.tile([128, 2, 1024], DT)

            qt_sb = [
                actpool.tile([128, TQ], DT, name=f"qt{hp}") for hp in range(HP)
            ]
            kt_sb = [
                actpool.tile([128, TK], DT, name=f"kt{hp}") for hp in range(HP)
            ]
            vn_sb = [
                actpool.tile([128, N_SC, 130], DT, name=f"vn{hp}")
                for hp in range(HP)
            ]
            ot_sb = [
                actpool.tile([128, TQ], DT, name=f"ot{hp}") for hp in range(HP)
            ]
            # softmax denominators on partition 0: (hp, h) at offset
            # (2*hp+h)*TQ; reciprocal'd in place (approx_fast), broadcast f32
            rec_all = actpool.tile([1, 4 * TQ], F32, name="rec_all")

            xfr_sb = actpool.tile([128, N_FC, TK], DT, name="xfr_sb")
            xto_sb = actpool.tile([128, N_FC, TQ], DT, name="xto_sb")

            # ---- DMA issue order: x_from stream first.  The big x loads
            # alternate between the two HWDGE queues (Sync + Scalar) so the
            # transfers run in parallel; small transfers go out on the
            # GpSimd SWDGE queue.
            nc.sync.dma_start(wk_sb[:], wk[:])
            nc.scalar.dma_start(wv_sb[:], wv[:])
            nc.gpsimd.dma_start(bk_sb[:], bk[:])
            nc.gpsimd.dma_start(bv_sb[:], bv[:])
            for fc in range(N_FC):
                q = nc.sync if fc % 2 == 0 else nc.scalar
                q.dma_start(xfr_sb[:, fc, :], xt_from_r[:, fc, :])
            nc.gpsimd.dma_start(bq_sb[:], bq[:])
            nc.scalar.dma_start(wq_sb[:], wq[:])
            for fc in range(N_FC):
                q = nc.sync if fc % 2 == 0 else nc.scalar
                q.dma_start(xto_sb[:, fc, :], xt_to_r[:, fc, :])
            nc.gpsimd.dma_start(wot_sb[:], wot[:])

            # ---- projection helpers --------------------------------------
            def proj_psum(tag):
                ps = pspool.tile([128, TT], F32, tag=tag, name=f"ps_{tag}")
                return ps

            def proj_fill_fc(ps, w_sb, x_sb, hp, tt, fc):
                dsl = bass.ts(hp, 128)
                for half in range(2):
                    nc.tensor.matmul(
                        ps[:, bass.ts(half, 512)],
                        w_sb[:, fc, dsl],
                        x_sb[:, fc, tt * TT + half * 512 : tt * TT + half * 512 + 512],
                        start=(fc == 0),
                        stop=(fc == N_FC - 1),
                    )

            def proj_cb_kq(ps, dst, b_sb, hp, tt):
                nc.vector.tensor_scalar_add(
                    dst[hp][:, bass.ts(tt, TT)], ps[:], b_sb[:, hp : hp + 1]
                )

            def proj_cb_v(ps, hp, tt):
                vtt = misc.tile([128, TT], DT, tag="vtt", name="vtt")
                nc.vector.tensor_scalar_add(vtt[:], ps[:], bv_sb[:, hp : hp + 1])
                return vtt

            def v_transpose(vtt, hp, tt, g):
                """Four 128x128 PE transposes into one PSUM tile, one
                strided DVE drain into vn (cols 0:64 -> 0:64, 64:128 ->
                65:129 per s-chunk)."""
                ps_t4 = pspool.tile([128, 512], DT, tag="s", name="ps_t4")
                for jj in range(4):
                    j = 4 * g + jj
                    nc.tensor.transpose(
                        ps_t4[:, bass.ts(jj, 128)], vtt[:, bass.ts(j, 128)],
                        ident[:],
                    )
                sc0 = tt * (TT // 128) + 4 * g
                dst = vn_sb[hp][:, sc0 : sc0 + 4, 0:130].rearrange(
                    "p s (g x) -> p s g x", g=2
                )
                src = ps_t4[:, :].rearrange("p (s g x) -> p s g x", s=4, g=2)
                nc.vector.tensor_copy(dst[:, :, :, 0:64], src[:, :, :, 0:64])

            # ---- Phase 1: K/V tt0 for both head pairs, streamed per fc ---
            ps_k0 = proj_psum("s")
            ps_v0 = proj_psum("s")
            ps_k1 = proj_psum("acc")
            ps_v1 = proj_psum("acc")
            for fc in range(N_FC):
                proj_fill_fc(ps_k0, wk_sb, xfr_sb, 0, 0, fc)
                proj_fill_fc(ps_v0, wv_sb, xfr_sb, 0, 0, fc)
                proj_fill_fc(ps_k1, wk_sb, xfr_sb, 1, 0, fc)
                proj_fill_fc(ps_v1, wv_sb, xfr_sb, 1, 0, fc)
            proj_cb_kq(ps_k0, kt_sb, bk_sb, 0, 0)
            vtt0 = proj_cb_v(ps_v0, 0, 0)
            proj_cb_kq(ps_k1, kt_sb, bk_sb, 1, 0)
            vtt1 = proj_cb_v(ps_v1, 1, 0)
            for g in range(2):
                v_transpose(vtt0, 0, 0, g)
            for g in range(2):
                v_transpose(vtt1, 1, 0, g)

            # ---- Phase 2: tt1 K/V for both head pairs, then Q(hp0,tt0);
            # the other three Q tiles stream in as stripe fillers
            ps_k0 = proj_psum("s")
            ps_v0 = proj_psum("s")
            ps_k1 = proj_psum("acc")
            ps_v1 = proj_psum("acc")
            for fc in range(N_FC):
                proj_fill_fc(ps_k0, wk_sb, xfr_sb, 0, 1, fc)
                proj_fill_fc(ps_v0, wv_sb, xfr_sb, 0, 1, fc)
                proj_fill_fc(ps_k1, wk_sb, xfr_sb, 1, 1, fc)
                proj_fill_fc(ps_v1, wv_sb, xfr_sb, 1, 1, fc)
            proj_cb_kq(ps_k0, kt_sb, bk_sb, 0, 1)
            vtt0 = proj_cb_v(ps_v0, 0, 1)
            proj_cb_kq(ps_k1, kt_sb, bk_sb, 1, 1)
            vtt1 = proj_cb_v(ps_v1, 1, 1)
            for g in range(2):
                v_transpose(vtt0, 0, 1, g)
            for g in range(2):
                v_transpose(vtt1, 1, 1, g)

            for hp in range(HP):
                nc.vector.memset(vn_sb[hp][:, :, 64], 1.0)
                nc.vector.memset(vn_sb[hp][:, :, 129], 1.0)

            ps_q = proj_psum("s")
            for fc in range(N_FC):
                proj_fill_fc(ps_q, wq_sb, xto_sb, 0, 0, fc)
            proj_cb_kq(ps_q, qt_sb, bq_sb, 0, 0)

            # ---- attention stripes ---------------------------------------
            # Fillers are (pe_heavy, thunk).  PE-heavy thunks are paced:
            # at most one per PE_SPACING s-chunks, so the matmul burst fits
            # in the per-chunk PE slack under the exp stream.  Non-PE
            # (DVE/GpSimd) thunks pop freely.
            from collections import deque

            fillers = deque()
            sc_clock = [0]
            last_pe_pop = [-10]
            PE_SPACING = 3
            BIG = 10**9

            def pop_fillers(sc):
                sc_clock[0] += 1
                budget = 3
                while fillers and budget > 0:
                    _deadline, pe_heavy, fn = fillers[0]
                    if pe_heavy:
                        # keep PE bursts away from the stripe tail so the
                        # next stripe's first score tiles aren't delayed
                        if (
                            sc_clock[0] - last_pe_pop[0] >= PE_SPACING
                            and sc < N_SC - 3
                        ):
                            fillers.popleft()
                            fn()
                            last_pe_pop[0] = sc_clock[0]
                            budget -= 2
                        else:
                            break
                    else:
                        fillers.popleft()
                        fn()
                        budget -= 1

            def force_deadline(stripe_idx):
                # correctness: everything a stripe consumes must be emitted
                # before the stripe's instructions are.  Pops from the front
                # (FIFO preserved) until no queued thunk violates.
                while any(d <= stripe_idx for d, _, _ in fillers):
                    fillers.popleft()[2]()

            def push_q_proj(hp, tt, deadline):
                # lives in the acc rotation: one multi-pop Q tile fits
                # between consecutive ps_o allocations without blocking
                # the score-psum ("s") rotation that paces the exp stream
                state = {}

                def fill(k):
                    if k == 0:
                        state["ps"] = pspool.tile(
                            [128, TT], F32, tag="acc", name="ps_qf"
                        )
                    for fc in (2 * k, 2 * k + 1):
                        proj_fill_fc(state["ps"], wq_sb, xto_sb, hp, tt, fc)

                def cb():
                    proj_cb_kq(state["ps"], qt_sb, bq_sb, hp, tt)

                for k in range(4):
                    fillers.append((deadline, True, lambda k=k: fill(k)))
                fillers.append((deadline, False, cb))

            def push_kv_proj(hp, tt, is_v, deadline):
                state = {}

                def fill(k):
                    if k == 0:
                        state["ps"] = pspool.tile(
                            [128, TT], F32, tag="acc", name="ps_kvf"
                        )
                    w = wv_sb if is_v else wk_sb
                    for fc in (2 * k, 2 * k + 1):
                        proj_fill_fc(state["ps"], w, xfr_sb, hp, tt, fc)

                def cb():
                    if is_v:
                        state["vtt"] = proj_cb_v(state["ps"], hp, tt)
                    else:
                        proj_cb_kq(state["ps"], kt_sb, bk_sb, hp, tt)

                for k in range(4):
                    fillers.append((deadline, True, lambda k=k: fill(k)))
                fillers.append((deadline, False, cb))
                if is_v:
                    for g in range(2):
                        fillers.append(
                            (deadline, True, lambda g=g, hp=hp, tt=tt: v_transpose(
                                state["vtt"], hp, tt, g
                            ))
                        )

            push_q_proj(0, 1, deadline=2)
            push_q_proj(1, 0, deadline=4)
            push_q_proj(1, 1, deadline=6)

            def rec_off(hp, tta, h):
                return ((hp * 4 + tta) * 2 + h) * TA

            pending_norms = []

            def emit_stripe(tta, hp, per_iter=1):
                force_deadline(hp * 4 + tta)
                # normalize the previous stripe's accumulator BEFORE this
                # stripe's PSUM writes are emitted — the acc slot may be
                # shared, and a later-emitted read of the dead tile races
                while pending_norms:
                    pending_norms.pop(0)()
                ps_o = pspool.tile([65, 1024], F32, tag="acc", name="ps_o")
                for sc in range(N_SC):
                    ps_s = pspool.tile([128, 1024], F32, tag="s", name="ps_s")
                    for h in range(2):
                        hb = 64 * h
                        nc.tensor.matmul(
                            ps_s[:, bass.ts(h, TA)],
                            kt_sb[hp][hb : hb + 64, bass.ts(sc, 128)],
                            qt_sb[hp][hb : hb + 64, bass.ts(tta, TA)],
                            start=True,
                            stop=True,
                        )
                    pt = ptpool.tile([128, 1024], DT, tag="pt", name="pt")
                    nc.scalar.activation(pt[:], ps_s[:], AF.Exp)
                    for h in range(2):
                        vb = 65 * h
                        nc.tensor.matmul(
                            ps_o[:, bass.ts(h, TA)],
                            vn_sb[hp][:, sc, vb : vb + 65],
                            pt[:, bass.ts(h, TA)],
                            start=(sc == 0),
                            stop=(sc == N_SC - 1),
                        )
                    pop_fillers(sc)
                # drain the accumulator with staging copies, then
                # reciprocal/broadcast/multiply as fillers during the next
                # stripe (the boundary DVE burst stays small)
                def drain_ot(h, ps_o=ps_o, hp=hp, tta=tta):
                    hb = 64 * h
                    nc.vector.tensor_copy(
                        ot_sb[hp][hb : hb + 64, bass.ts(tta, TA)],
                        ps_o[0:64, bass.ts(h, TA)],
                    )

                def drain_rec(ps_o=ps_o, hp=hp, tta=tta):
                    base = rec_off(hp, tta, 0)
                    nc.vector.tensor_copy(
                        rec_all[0:1, base : base + 2 * TA],
                        ps_o[64:65, :],
                    )

                def norm(h, hp=hp, tta=tta):
                    off = rec_off(hp, tta, h)
                    nc.vector.reciprocal_approx_fast(
                        rec_all[0:1, off : off + TA],
                        rec_all[0:1, off : off + TA],
                    )
                    r_sb = misc.tile([128, TA], F32, tag="rsb", name="r_sb")
                    nc.gpsimd.partition_broadcast(
                        r_sb[:], rec_all[0:1, off : off + TA]
                    )
                    hb = 64 * h
                    nc.vector.tensor_mul(
                        ot_sb[hp][hb : hb + 64, bass.ts(tta, TA)],
                        ot_sb[hp][hb : hb + 64, bass.ts(tta, TA)],
                        r_sb[hb : hb + 64, :],
                    )

                pending_norms.append(lambda: drain_ot(0))
                pending_norms.append(drain_rec)
                pending_norms.append(lambda: drain_ot(1))
                fillers.append((BIG, False, lambda: norm(0)))
                fillers.append((BIG, False, lambda: norm(1)))

            def push_outproj(tta, last=False):
                for j in range(TA // 128):
                    tc_ = tta * (TA // 128) + j

                    def thunk(tc_=tc_):
                        tsl = bass.ts(tc_, 128)
                        ps_out = pspool.tile(
                            [128, 1024], F32, tag="s", name="ps_out"
                        )
                        for half in range(2):
                            hsl = bass.ts(half, 512)
                            for hp in range(HP):
                                nc.tensor.matmul(
                                    ps_out[:, hsl],
                                    ot_sb[hp][:, tsl],
                                    wot_sb[:, hp, hsl],
                                    start=(hp == 0),
                                    stop=(hp == HP - 1),
                                )
                        o_t = misc.tile(
                            [128, 1024], DT, tag="out", name="o_t", bufs=3
                        )
                        if tc_ % 2 == 1:
                            # the exp gap each outproj pop creates absorbs
                            # an ACT drain; alternating also halves the DVE
                            # load so the "s" slot frees faster
                            nc.scalar.activation(o_t[:], ps_out[:], AF.Copy)
                        else:
                            nc.vector.tensor_copy(o_t[:], ps_out[:])
                        nc.sync.dma_start(out[tsl, :], o_t[:])

                    fillers.append((BIG, True, thunk))

            for hp in range(HP):
                for tta in range(TQ // TA):
                    emit_stripe(tta, hp, per_iter=1)
                    if hp == 1:
                        push_outproj(tta, last=(tta == TQ // TA - 1))

            while pending_norms:
                pending_norms.pop(0)()
            while fillers:
                fillers.popleft()[2]()

    nc.compile()
    return nc


def _prep_in_maps(x_to, x_from, Wq, bq, Wk, bk, Wv, bv, Wo):
    scale = 1.0 / np.sqrt(np.float32(DH))
    # [H, D, DH] -> [D, H*DH] with column h*DH+d
    wq_f = np.ascontiguousarray(Wq.transpose(1, 0, 2).reshape(D, H * DH)) * scale
    wk_f = np.ascontiguousarray(Wk.transpose(1, 0, 2).reshape(D, H * DH))
    wv_f = np.ascontiguousarray(Wv.transpose(1, 0, 2).reshape(D, H * DH))
    bq_f = bq.reshape(H * DH) * scale
    bk_f = bk.reshape(H * DH)
    bv_f = bv.reshape(H * DH)

    xt_to = np.ascontiguousarray(x_to.transpose(0, 2, 1))    # [B, D, TQ]
    xt_from = np.ascontiguousarray(x_from.transpose(0, 2, 1))

    def f32(a):
        return np.ascontiguousarray(a, dtype=np.float32)

    import ml_dtypes

    def fdt(a):
        return np.ascontiguousarray(a, dtype=ml_dtypes.bfloat16)

    in_maps = []
    for c in range(N_CORES):
        b, g = divmod(c, HEADS_PER_CORE)
        cs = slice(g * 256, (g + 1) * 256)
        in_maps.append(
            {
                "xt_to": fdt(xt_to[b]),
                "xt_from": fdt(xt_from[b]),
                # [D, 256] -> [c, p, d] -> [p, c, d] (4KB/partition DMA)
                "wq": fdt(wq_f[:, cs].reshape(N_FC, 128, 256).transpose(1, 0, 2)),
                "wk": fdt(wk_f[:, cs].reshape(N_FC, 128, 256).transpose(1, 0, 2)),
                "wv": fdt(wv_f[:, cs].reshape(N_FC, 128, 256).transpose(1, 0, 2)),
                # [256] -> [2 pairs, 128] -> [128, 2]
                "bq": f32(bq_f[cs].reshape(2, 128).T),
                "bk": f32(bk_f[cs].reshape(2, 128).T),
                "bv": f32(bv_f[cs].reshape(2, 128).T),
                # Wo[:, cs].T = [256, 1024] -> [2, 128, 1024] -> [128, 2, 1024]
                "wot": fdt(
                    np.ascontiguousarray(Wo[:, cs].T)
                    .reshape(2, 128, 1024)
                    .transpose(1, 0, 2)
                ),
            }
        )
    return in_maps


LAST_EXEC_TIME_NS = None
LAST_TRACE = None


def kernel(x_to, x_from, Wq, bq, Wk, bk, Wv, bv, Wo, bo):
    global LAST_EXEC_TIME_NS, LAST_TRACE
    if "nc" not in _CACHED:
        _CACHED["nc"] = build_program()
    nc = _CACHED["nc"]

    in_maps = _prep_in_maps(
        np.asarray(x_to), np.asarray(x_from), np.asarray(Wq), np.asarray(bq),
        np.asarray(Wk), np.asarray(bk), np.asarray(Wv), np.asarray(bv),
        np.asarray(Wo),
    )
    res = run_bass_kernel_spmd(nc, in_maps, list(range(N_CORES)))
    LAST_EXEC_TIME_NS = res.exec_time_ns
    LAST_TRACE = res.instructions_and_trace

    out = np.zeros((B, TQ, D), dtype=np.float32)
    for c in range(N_CORES):
        out[c // HEADS_PER_CORE] += np.asarray(
            res.results[c]["out"], dtype=np.float32
        )
    out += np.asarray(bo, dtype=np.float32)
    return out
